# revision 1
# baseline (speedup 1.0000x reference)
"""Trainium2 Bass kernel for nn_CrossModalAttention.

Reference computation (per token t of B*N tokens):
  x = [x_tech_t; x_sent_t; x_fin_t]            # [3, 256]
  q/k/v = x @ W{q,k,v} + b                     # [3, 4, 64]
  scores = q k^T / 8 (per head), softmax over j
  ctx = attn @ v; attn_out = ctx @ Wo + bo     # [3, 256]
  y = x + attn_out; LayerNorm(d) per slot; mean over 3 slots -> [256]

Sharding: pure data-parallel over batch (64 -> 8 per core x 8 cores).

Per-core dataflow (TOK tokens, super-tiles of 512 = 4 sub-tiles of 128):
  - gpsimd cast-DMA HBM fp32 -> SBUF bf16, token-major xb [128,4,256]
  - PE transposes (identity matmul) -> xT feature-major [128,2,512]
  - Q,K: PE W-stationary -> feature-major psum; evac bf16 (ACT/DVE)
  - V: PE X^T-stationary -> token-major psum directly; evac bf16
  - scores: DVE/gpsimd mul P=Q_i^T*K_j^T; PE segment-reduce (indicator
    matmuls, 1/8 folded in) -> scores psum [96,512] rows=(j,i,h) 32-aligned
  - softmax: ACT exp; Z via PE indicator matmul; 1/Z = ACT exp(-ln Z);
    replicate via PE matmul; one DVE mul
  - a -> token-major via DMA-xbar transpose [128,4,128]
  - ctx: DVE/gpsimd tensor_tensor with 0-step free-dim broadcast of a over k
  - ctx -> PE-transpose -> ctxT; O-proj PE ctxT-stationary -> token-major psum
  - residual+LN: ACT evac, gpsimd residual add, DVE bn_stats/bn_aggr,
    istd via ACT Ln/Exp (exp table set shared), apply via tensor_scalar,
    slot-mean folded into istd (x 1/3)
"""

import numpy as np

D = 256
H = 4
KD = 64
EPS = 1e-6
B, N = 64, 1024
NCORES = 8
ST = 512          # tokens per super-tile
SUB = 4           # 128-token sub-tiles per super-tile
P = 128

_CACHE = {}
REPEAT = 1       # timing knob: loop the per-core program this many times
XT_FROM_DRAM = True   # False: PE-transpose xT on device
NULL_KERNEL = False   # timing: emit only output writes (dispatch baseline)
SHRINK = set()        # timing ablation: stages replaced by cheap memsets
_RUN_KWARGS = {}   # test harness may set e.g. {"trace": True}
_LAST_RESULT = [None]


def _build(TOK, use_qkv_bias, use_bo, use_gamma, use_beta):
    import concourse.bass as bass
    import concourse.bacc as bacc
    import concourse.mybir as mybir
    import concourse.tile as tile

    fp32 = mybir.dt.float32
    bf16 = mybir.dt.bfloat16
    AF = mybir.ActivationFunctionType
    OP = mybir.AluOpType

    nst = TOK // ST
    assert TOK % ST == 0

    nc = bacc.Bacc("TRN2", target_bir_lowering=False)

    # ---- DRAM I/O ----
    xb_d = nc.dram_tensor("xb_pre", [3, TOK, D], bf16, kind="ExternalInput")
    xt_d = nc.dram_tensor("xt_pre", [3, 2, P, TOK], bf16, kind="ExternalInput")
    wqkv_d = nc.dram_tensor("wqkv", [P, 2, 3 * D], bf16, kind="ExternalInput")
    wo_d = nc.dram_tensor("wo", [P, 2, D], bf16, kind="ExternalInput")
    seg_d = nc.dram_tensor("seg", [P, 2, 3, 3, 96], bf16, kind="ExternalInput")
    jsum_d = nc.dram_tensor("jsum", [P, 32], bf16, kind="ExternalInput")
    jrep_d = nc.dram_tensor("jrep", [32, P], fp32, kind="ExternalInput")
    iden_d = nc.dram_tensor("iden", [P, P], bf16, kind="ExternalInput")
    bqkv_d = nc.dram_tensor("bqkv", [P, 6], fp32, kind="ExternalInput")
    bo_d = nc.dram_tensor("bo_t", [1, D], fp32, kind="ExternalInput")
    gam_d = nc.dram_tensor("gam_t", [1, D], bf16, kind="ExternalInput")
    bet_d = nc.dram_tensor("bet_t", [1, D], bf16, kind="ExternalInput")
    out_d = nc.dram_tensor("out", [TOK, D], fp32, kind="ExternalOutput")

    with tile.TileContext(nc) as tc:
        with tc.tile_pool(name="const", bufs=1) as constp, \
             tc.tile_pool(name="ld", bufs=3) as ldp, \
             tc.tile_pool(name="qk", bufs=3) as qkp, \
             tc.tile_pool(name="mid", bufs=3) as midp, \
             tc.tile_pool(name="small", bufs=3) as smallp, \
             tc.tile_pool(name="ctxp", bufs=3) as ctxp, \
             tc.tile_pool(name="lnp", bufs=2) as lnp, \
             tc.tile_pool(name="qk_ps", bufs=2, space="PSUM") as qk_ps, \
             tc.tile_pool(name="vo_ps", bufs=2, space="PSUM") as vo_ps, \
             tc.tile_pool(name="sc_ps", bufs=2, space="PSUM") as sc_psp, \
             tc.tile_pool(name="tp_ps", bufs=2, space="PSUM") as tp_ps:

            # ---- constants ----
            wqkv = constp.tile([P, 2, 3 * D], bf16)
            nc.sync.dma_start(out=wqkv, in_=wqkv_d[:])
            wo = constp.tile([P, 2, D], bf16)
            nc.sync.dma_start(out=wo, in_=wo_d[:])
            seg = constp.tile([P, 2, 3, 3, 96], bf16)
            nc.sync.dma_start(out=seg, in_=seg_d[:])
            jsum = constp.tile([P, 32], bf16)
            nc.sync.dma_start(out=jsum, in_=jsum_d[:])
            jrep = constp.tile([32, P], fp32)
            nc.sync.dma_start(out=jrep, in_=jrep_d[:])
            iden = constp.tile([P, P], bf16)
            nc.sync.dma_start(out=iden, in_=iden_d[:])
            bqkv = constp.tile([P, 6], fp32)
            nc.sync.dma_start(out=bqkv, in_=bqkv_d[:])
            if use_bo:
                bo_rep = constp.tile([P, 2, D], fp32)
                nc.sync.dma_start(out=bo_rep,
                                  in_=bo_d[:].to_broadcast((P, 2, D)))
            eps_c = constp.tile([P, 1], fp32)
            nc.vector.memset(eps_c, EPS)
            mln3_c = constp.tile([P, 1], fp32)
            nc.vector.memset(mln3_c, -float(np.log(3.0)))
            if use_gamma:
                gam = constp.tile([P, D], bf16)
                nc.sync.dma_start(out=gam, in_=gam_d[:].to_broadcast((P, D)))
            if use_beta:
                bet = constp.tile([P, D], bf16)
                nc.sync.dma_start(out=bet, in_=bet_d[:].to_broadcast((P, D)))

            # greedy busy-tracking engine balancer (ns estimates)
            load = {"act": 0.0, "dve": 0.0, "pool": 0.0}

            def evac(dst, src, fd):
                # psum -> sbuf copy: ACT (fd+352)/1.2 vs DVE (120+fd/2)/0.96
                ca = (fd + 352) / 1.2
                cd = (120 + fd / 2) / 0.96
                if load["act"] + ca <= load["dve"] + cd:
                    load["act"] += ca
                    nc.scalar.copy(out=dst, in_=src)
                else:
                    load["dve"] += cd
                    nc.vector.tensor_copy(out=dst, in_=src)

            def tt(out, in0, in1, op, fd, psum=False):
                # bf16 TT: DVE 2x vs gpsimd ~1x (sbuf only)
                cd = ((120 if psum else 58) + fd / 2) / 0.96
                cp = (58 + fd) / 1.2
                if psum or load["dve"] + cd <= load["pool"] + cp:
                    load["dve"] += cd
                    nc.vector.tensor_tensor(out=out, in0=in0, in1=in1, op=op)
                else:
                    load["pool"] += cp
                    nc.gpsimd.tensor_tensor(out=out, in0=in0, in1=in1, op=op)

            def ts2(out, in0, s1, s2, fd):
                cd = (58 + fd / 4) / 0.96
                cp = (58 + fd / 2) / 1.2
                if load["dve"] + cd <= load["pool"] + cp:
                    load["dve"] += cd
                    nc.vector.tensor_scalar(out=out, in0=in0, scalar1=s1,
                                            scalar2=s2, op0=OP.subtract,
                                            op1=OP.mult)
                else:
                    load["pool"] += cp
                    nc.gpsimd.tensor_scalar(out=out, in0=in0, scalar1=s1,
                                            scalar2=s2, op0=OP.subtract,
                                            op1=OP.mult)

            def pe_transpose4(dst4, srcs):
                # 4x [128,128] transposes into one psum bank, single evac
                tp = tp_ps.tile([P, SUB, P], bf16, tag="tp")
                for s, sl in enumerate(srcs):
                    nc.tensor.transpose(tp[:, s, :], sl, iden)
                evac(dst4, tp, SUB * P)

            if NULL_KERNEL:
                zt = constp.tile([P, SUB, D], fp32)
                nc.vector.memset(zt, 0.0)
                for st in range(nst):
                    t0 = st * ST
                    dstn = out_d[t0:t0 + ST, :].rearrange("(s p) d -> p s d",
                                                          p=P)
                    nc.sync.dma_start(out=dstn, in_=zt)
                nst = 0
            for _rep in range(REPEAT):
              for st in range(nst):
                t0 = st * ST
                # ---------- load + cast + PE-transpose ----------
                xb = []    # token-major bf16 [128, SUB, 256]
                xT = []    # feature-major bf16 [128, 2, 512]
                for i in range(3):
                    xbi = ldp.tile([P, SUB, D], bf16, tag=f"xb{i}")
                    src = xb_d[i, t0:t0 + ST, :].rearrange(
                        "(s p) d -> p s d", p=P)
                    nc.sync.dma_start(out=xbi, in_=src)
                    xb.append(xbi)
                    xTi = ldp.tile([P, 2, ST], bf16, tag=f"xT{i}")
                    if XT_FROM_DRAM:
                        nc.sync.dma_start(
                            out=xTi,
                            in_=xt_d[i, :, :, t0:t0 + ST].rearrange(
                                "c p t -> p c t"))
                    else:
                        for c in range(2):
                            pe_transpose4(
                                xTi[:, c, :],
                                [xbi[:, s, c * P:(c + 1) * P]
                                 for s in range(SUB)])
                    xT.append(xTi)

                # ---------- Q,K (W-stationary, feature-major) ----------
                qT, kT = [], []
                for i in range(3):
                    for pj in range(2):  # 0=q 1=k
                        dst = qkp.tile([P, 2, ST], bf16, tag=f"p{pj}m{i}")
                        for m in range(2):
                            ps = qk_ps.tile([P, ST], fp32, tag="qkps")
                            for c in range(2):
                                nc.tensor.matmul(
                                    ps,
                                    lhsT=wqkv[:, c,
                                              pj * D + m * P: pj * D + (m + 1) * P],
                                    rhs=xT[i][:, c, :],
                                    start=(c == 0), stop=(c == 1))
                            if 'evacqk' in SHRINK:
                                nc.vector.memset(dst[:, m, :], 0.1)
                            elif use_qkv_bias:
                                nc.scalar.activation(
                                    out=dst[:, m, :], in_=ps,
                                    func=AF.Identity,
                                    bias=bqkv[:, pj * 2 + m: pj * 2 + m + 1])
                            else:
                                evac(dst[:, m, :], ps, ST)
                        (qT if pj == 0 else kT).append(dst)

                # ---------- V (X^T-stationary, token-major) ----------
                vtok = []
                for i in range(3):
                    vt = midp.tile([P, SUB, D], bf16, tag=f"vtok{i}")
                    for spair in range(2):  # two sub-tiles per psum bank
                        ps = vo_ps.tile([P, 2, D], fp32, tag="vps")
                        for shalf in range(2):
                            s = spair * 2 + shalf
                            for c in range(2):
                                nc.tensor.matmul(
                                    ps[:, shalf, :],
                                    lhsT=xT[i][:, c, s * P:(s + 1) * P],
                                    rhs=wqkv[:, c, 2 * D:3 * D],
                                    start=(c == 0), stop=(c == 1))
                        evac(vt[:, spair * 2:spair * 2 + 2, :], ps, 2 * D)
                    vtok.append(vt)

                # ---------- scores ----------
                scp = sc_psp.tile([96, ST], fp32, tag="scmix")
                first = True
                for j in range(3):
                    for i in range(3):
                        pt = smallp.tile([P, 2, ST], bf16, tag="pmul")
                        if 'pmul' in SHRINK:
                            nc.vector.memset(pt, 0.25)
                        else:
                            tt(pt, qT[i], kT[j], OP.mult, 2 * ST)
                        for m in range(2):
                            last = (j == 2 and i == 2 and m == 1)
                            if 'seg' in SHRINK:
                                first = False
                                continue
                            nc.tensor.matmul(
                                scp, lhsT=seg[:, m, j, i, :], rhs=pt[:, m, :],
                                start=first, stop=last,
                                skip_group_check=True)
                            first = False
                if 'seg' in SHRINK:
                    nc.tensor.matmul(scp, lhsT=seg[:, 0, 0, 0, :],
                                     rhs=pt[:, 0, :], start=True, stop=True)

                # ---------- softmax ----------
                es = smallp.tile([P, ST], bf16, tag="es")
                nc.gpsimd.memset(es[96:128, :], 0.0)
                nc.scalar.activation(out=es[0:96, :], in_=scp[0:96, :],
                                     func=AF.Exp)
                zps = sc_psp.tile([32, ST], fp32, tag="scmix")
                nc.tensor.matmul(zps, lhsT=jsum[0:96, :], rhs=es[0:96, :],
                                 start=True, stop=True)
                zi = smallp.tile([32, ST], fp32, tag="zi")
                lnz = smallp.tile([32, ST], fp32, tag="lnz")
                nc.scalar.activation(out=lnz, in_=zps, func=AF.Ln)
                nc.scalar.activation(out=zi, in_=lnz, func=AF.Exp, scale=-1.0)
                zr = sc_psp.tile([P, ST], fp32, tag="scmix")
                nc.tensor.matmul(zr, lhsT=jrep, rhs=zi, start=True, stop=True)
                asb = smallp.tile([P, ST], bf16, tag="asb")
                tt(asb, es, zr, OP.mult, ST, psum=True)
                aT = smallp.tile([P, SUB, P], bf16, tag="aT")
                for s in range(SUB):
                    nc.sync.dma_start(out=aT[:, s, :],
                                      in_=asb[:, s * P:(s + 1) * P],
                                      transpose=True)

                # ---------- ctx ----------
                ctxT = []
                for i in range(3):
                    cx = ctxp.tile([P, SUB, D], bf16, tag=f"cx{i}")
                    tmp = ctxp.tile([P, SUB, D], bf16, tag="cxtmp")
                    cx4 = cx.rearrange("p s (h k) -> p s h k", h=H)
                    tmp4 = tmp.rearrange("p s (h k) -> p s h k", h=H)
                    if 'ctx' in SHRINK:
                        nc.vector.memset(cx, 0.5)
                    else:
                      for j in range(3):
                        asl = aT[:, :, 32 * j + 4 * i: 32 * j + 4 * i + 4]
                        abc = bass.AP(tensor=asl.tensor, offset=asl.offset,
                                      ap=[*asl.ap, [0, KD]])
                        v4 = vtok[j].rearrange("p s (h k) -> p s h k", h=H)
                        dst = cx4 if j == 0 else tmp4
                        tt(dst, v4, abc, OP.mult, SUB * D)
                        if j > 0:
                            tt(cx4, cx4, tmp4, OP.add, SUB * D)
                    cT = ctxp.tile([P, 2, ST], bf16, tag=f"cT{i}")
                    if 'ctxT' in SHRINK:
                        nc.vector.memset(cT, 0.2)
                    else:
                        for c in range(2):
                            pe_transpose4(
                                cT[:, c, :],
                                [cx[:, s, c * P:(c + 1) * P]
                                 for s in range(SUB)])
                    ctxT.append(cT)

                # ---------- O-proj (ctxT-stationary, token-major) + LN ------
                mvs = lnp.tile([P, 12, 2], fp32, tag="mvs")
                ys = []
                for i in range(3):
                    yi = lnp.tile([P, SUB, D], bf16, tag=f"y{i}")
                    for spair in range(2):
                        ops = vo_ps.tile([P, 2, D], fp32, tag="vps")
                        for shalf in range(2):
                            s = spair * 2 + shalf
                            for c in range(2):
                                nc.tensor.matmul(
                                    ops[:, shalf, :],
                                    lhsT=ctxT[i][:, c, s * P:(s + 1) * P],
                                    rhs=wo[:, c, :],
                                    start=(c == 0), stop=(c == 1))
                        if use_bo:
                            nc.vector.tensor_tensor(
                                out=ops, in0=ops, in1=bo_rep, op=OP.add)
                        ao = lnp.tile([P, 2, D], bf16, tag="ao")
                        evac(ao, ops, 2 * D)
                        for shalf in range(2):
                            s = spair * 2 + shalf
                            idx = i * SUB + s
                            if 'ln' in SHRINK:
                                continue
                            tt(yi[:, s, :], xb[i][:, s, :], ao[:, shalf, :],
                               OP.add, D)
                            st6 = lnp.tile([P, 6], fp32, tag="st6")
                            nc.vector.bn_stats(out=st6, in_=yi[:, s, :])
                            nc.vector.bn_aggr(out=mvs[:, idx, :], in_=st6)
                    ys.append(yi)

                # ---------- stats -> mu, istd/3 ----------
                if 'ln' in SHRINK:
                    otok = lnp.tile([P, SUB, D], fp32, tag="otok")
                    nc.vector.memset(otok, 0.0)
                    dst = out_d[t0:t0 + ST, :].rearrange("(s p) d -> p s d",
                                                         p=P)
                    nc.gpsimd.dma_start(out=dst, in_=otok)
                    continue
                lnv = lnp.tile([P, 12], fp32, tag="lnv")
                nc.scalar.activation(out=lnv, in_=mvs[:, :, 1], func=AF.Ln,
                                     bias=eps_c)
                ist = lnp.tile([P, 12], fp32, tag="ist")
                nc.scalar.activation(out=ist, in_=lnv, func=AF.Exp,
                                     scale=-0.5, bias=mln3_c)

                # ---------- apply + slot mean + store ----------
                otok = lnp.tile([P, SUB, D], fp32, tag="otok")
                for s in range(SUB):
                    n0 = lnp.tile([P, D], bf16, tag="n0")
                    n01 = lnp.tile([P, D], bf16, tag="n01")
                    n2 = lnp.tile([P, D], bf16, tag="n2")
                    idx = lambda i: i * SUB + s  # noqa: E731
                    ts2(n0, ys[0][:, s, :], mvs[:, idx(0), 0:1],
                        ist[:, idx(0):idx(0) + 1], D)
                    ts2(n2, ys[1][:, s, :], mvs[:, idx(1), 0:1],
                        ist[:, idx(1):idx(1) + 1], D)
                    tt(n01, n0, n2, OP.add, D)
                    ts2(n2, ys[2][:, s, :], mvs[:, idx(2), 0:1],
                        ist[:, idx(2):idx(2) + 1], D)
                    if use_gamma or use_beta:
                        fse = lnp.tile([P, D], bf16, tag="fse")
                        nc.vector.tensor_tensor(out=fse, in0=n01, in1=n2,
                                                op=OP.add)
                        if use_gamma:
                            nc.vector.tensor_tensor(out=fse, in0=fse, in1=gam,
                                                    op=OP.mult)
                        if use_beta:
                            nc.vector.tensor_tensor(out=otok[:, s, :], in0=fse,
                                                    in1=bet, op=OP.add)
                        else:
                            nc.vector.tensor_copy(out=otok[:, s, :], in_=fse)
                    else:
                        tt(otok[:, s, :], n01, n2, OP.add, D)
                dst = out_d[t0:t0 + ST, :].rearrange("(s p) d -> p s d", p=P)
                nc.gpsimd.dma_start(out=dst, in_=otok)

    nc.compile()
    return nc


def _prep_weights(Wq, bq, Wk, bk, Wv, bv, Wo, bo, gamma, beta):
    """Host-side packing of the small parameter tensors."""
    import ml_dtypes
    Wq2 = Wq.reshape(D, D)            # [d, (h k)]
    Wk2 = Wk.reshape(D, D)
    Wv2 = Wv.reshape(D, D)
    Wcat = np.concatenate([Wq2, Wk2, Wv2], axis=1)       # [256, 768]
    wqkv = np.ascontiguousarray(
        Wcat.reshape(2, P, 3 * D).transpose(1, 0, 2))     # [128, 2, 768]
    Wo2 = Wo.reshape(D, D)                                # [(h k), d]
    wo = np.ascontiguousarray(Wo2.reshape(2, P, D).transpose(1, 0, 2))
    seg = np.zeros((P, 2, 3, 3, 96), np.float32)
    for m in range(2):
        for p in range(P):
            h = (m * P + p) // KD
            for j in range(3):
                for i in range(3):
                    seg[p, m, j, i, 32 * j + 4 * i + h] = 0.125
    jsum = np.zeros((P, 32), np.float32)
    for p in range(96):
        jsum[p, p % 32] = 1.0
    jrep = np.zeros((32, P), np.float32)
    for p in range(P):
        jrep[p % 32, p] = 1.0
    bcat = np.concatenate([bq.reshape(D), bk.reshape(D), bv.reshape(D)])
    bqkv = np.ascontiguousarray(bcat.reshape(3, 2, P).transpose(2, 0, 1)
                                .reshape(P, 6)).astype(np.float32)
    # v-bias folds into an effective output bias since softmax rows sum to 1:
    # ctx = sum_j a_ij (v_j + bv) = (sum_j a_ij v_j) + bv  ->  bv @ Wo + bo
    bo_eff = (bv.reshape(D) @ Wo.reshape(D, D) + bo.reshape(D))
    to_bf = lambda a: a.astype(ml_dtypes.bfloat16)  # noqa: E731
    return {
        "wqkv": to_bf(wqkv), "wo": to_bf(wo), "seg": to_bf(seg),
        "bqkv": bqkv, "bo_t": bo_eff.reshape(1, D).astype(np.float32),
        "jsum": to_bf(jsum), "jrep": jrep.astype(np.float32),
        "iden": to_bf(np.eye(P, dtype=np.float32)),
        "gam_t": to_bf(gamma.reshape(1, D)), "bet_t": to_bf(beta.reshape(1, D)),
    }


def kernel(**inputs):
    from concourse.bass_utils import run_bass_kernel_spmd

    xs = {k: np.asarray(inputs[k], np.float32)
          for k in ("x_tech", "x_sent", "x_fin")}
    params = {k: np.asarray(inputs[k], np.float32) for k in
              ("Wq", "bq", "Wk", "bk", "Wv", "bv", "Wo", "bo", "gamma", "beta")}

    use_qkv_bias = any(np.any(params[b]) for b in ("bq", "bk", "bv"))
    use_bo = bool(np.any(params["bo"])) or bool(np.any(params["bv"]))
    use_gamma = bool(np.any(params["gamma"] != 1.0))
    use_beta = bool(np.any(params["beta"]))

    TOK = (B // NCORES) * N
    key = (TOK, use_qkv_bias, use_bo, use_gamma, use_beta)
    if key not in _CACHE:
        _CACHE[key] = _build(*key)
    nc = _CACHE[key]

    wmap = _prep_weights(**params)
    import ml_dtypes
    in_maps = []
    for c in range(NCORES):
        m = dict(wmap)
        xt = np.empty((3, 2, P, TOK), ml_dtypes.bfloat16)
        xbp = np.empty((3, TOK, D), ml_dtypes.bfloat16)
        for ii, name in enumerate(("x_tech", "x_sent", "x_fin")):
            sl = xs[name][c * (B // NCORES):(c + 1) * (B // NCORES)]
            flat = np.ascontiguousarray(sl.reshape(TOK, D))
            xbp[ii] = flat.astype(ml_dtypes.bfloat16)
            xt[ii] = flat.T.reshape(2, P, TOK).astype(ml_dtypes.bfloat16)
        m["xt_pre"] = xt
        m["xb_pre"] = xbp
        in_maps.append(m)

    res = run_bass_kernel_spmd(nc, in_maps, core_ids=list(range(NCORES)),
                               **_RUN_KWARGS)
    _LAST_RESULT[0] = res
    out = np.stack([r["out"].reshape(B // NCORES, N, D)
                    for r in res.results])
    return out.reshape(B, N, D).astype(np.float32)



# revision 18
# speedup vs baseline: 2.7718x; 2.7718x over previous
"""Trainium2 Bass kernel for nn_CrossModalAttention.

Reference computation (per token t of B*N tokens):
  x = [x_tech_t; x_sent_t; x_fin_t]            # [3, 256]
  q/k/v = x @ W{q,k,v} + b                     # [3, 4, 64]
  scores = q k^T / 8 (per head), softmax over j
  ctx = attn @ v; attn_out = ctx @ Wo + bo     # [3, 256]
  y = x + attn_out; LayerNorm(d) per slot; mean over 3 slots -> [256]

Sharding: pure data-parallel over batch (64 -> 8 per core x 8 cores).

Per-core dataflow (TOK tokens, super-tiles of 512 = 4 sub-tiles of 128):
  - gpsimd cast-DMA HBM fp32 -> SBUF bf16, token-major xb [128,4,256]
  - PE transposes (identity matmul) -> xT feature-major [128,2,512]
  - Q,K: PE W-stationary -> feature-major psum; evac bf16 (ACT/DVE)
  - V: PE X^T-stationary -> token-major psum directly; evac bf16
  - scores: DVE/gpsimd mul P=Q_i^T*K_j^T; PE segment-reduce (indicator
    matmuls, 1/8 folded in) -> scores psum [96,512] rows=(j,i,h) 32-aligned
  - softmax: ACT exp; Z via PE indicator matmul; 1/Z = ACT exp(-ln Z);
    replicate via PE matmul; one DVE mul
  - a -> token-major via DMA-xbar transpose [128,4,128]
  - ctx: DVE/gpsimd tensor_tensor with 0-step free-dim broadcast of a over k
  - ctx -> PE-transpose -> ctxT; O-proj PE ctxT-stationary -> token-major psum
  - residual+LN: ACT evac, gpsimd residual add, DVE bn_stats/bn_aggr,
    istd via ACT Ln/Exp (exp table set shared), apply via tensor_scalar,
    slot-mean folded into istd (x 1/3)
"""

import numpy as np

D = 256
H = 4
KD = 64
EPS = 1e-6
B, N = 64, 1024
NCORES = 8
ST = 512          # tokens per super-tile
SUB = 4           # 128-token sub-tiles per super-tile
P = 128

_CACHE = {}
REPEAT = 1       # timing knob: loop the per-core program this many times
XT_FROM_DRAM = False  # False: PE-transpose xT on device
NULL_KERNEL = False   # timing: emit only output writes (dispatch baseline)
SHRINK = set()        # timing ablation: stages replaced by cheap memsets
_RUN_KWARGS = {}   # test harness may set e.g. {"trace": True}
_LAST_RESULT = [None]


def _build(TOK, use_qkv_bias, use_bo, use_gamma, use_beta):
    import concourse.bass as bass
    import concourse.bacc as bacc
    import concourse.mybir as mybir
    import concourse.tile as tile

    fp32 = mybir.dt.float32
    bf16 = mybir.dt.bfloat16
    AF = mybir.ActivationFunctionType
    OP = mybir.AluOpType

    nst = TOK // ST
    assert TOK % ST == 0

    nc = bacc.Bacc("TRN2", target_bir_lowering=False)

    # ---- DRAM I/O ----
    xb_d = nc.dram_tensor("xb_pre", [3, TOK, D], bf16, kind="ExternalInput")
    if XT_FROM_DRAM:
        xt_d = nc.dram_tensor("xt_pre", [3, 2, P, TOK], bf16,
                              kind="ExternalInput")
    # all bf16 consts packed into one tensor (fewer transfers):
    #   cols 0:1536 wqkv [P,2,768] | 1536:2048 wo [P,2,256]
    #   | 2048:3776 seg [P,2,3,3,96] | 3776:3808 jsum | 3808:3936 iden
    cb16_d = nc.dram_tensor("cb16", [P, 3936], bf16, kind="ExternalInput")
    # fp32 consts: cols 0:128 jrep (rows 0:32) | 128:134 bqkv
    cb32_d = nc.dram_tensor("cb32", [P, 134], fp32, kind="ExternalInput")
    if use_bo:
        bo_d = nc.dram_tensor("bo_t", [1, D], fp32, kind="ExternalInput")
    if use_gamma:
        gam_d = nc.dram_tensor("gam_t", [1, D], bf16, kind="ExternalInput")
    if use_beta:
        bet_d = nc.dram_tensor("bet_t", [1, D], bf16, kind="ExternalInput")
    out_d = nc.dram_tensor("out", [TOK, D], bf16, kind="ExternalOutput")

    with tile.TileContext(nc) as tc:
        with tc.tile_pool(name="const", bufs=1) as constp, \
             tc.tile_pool(name="ld", bufs=3) as ldp, \
             tc.tile_pool(name="qk", bufs=3) as qkp, \
             tc.tile_pool(name="mid", bufs=3) as midp, \
             tc.tile_pool(name="small", bufs=3) as smallp, \
             tc.tile_pool(name="ctxp", bufs=3) as ctxp, \
             tc.tile_pool(name="lnp", bufs=2) as lnp, \
             tc.tile_pool(name="qk_ps", bufs=2, space="PSUM") as qk_ps, \
             tc.tile_pool(name="vo_ps", bufs=2, space="PSUM") as vo_ps, \
             tc.tile_pool(name="sc_ps", bufs=2, space="PSUM") as sc_psp, \
             tc.tile_pool(name="tp_ps", bufs=2, space="PSUM") as tp_ps:

            # ---- constants (packed loads + views) ----
            cb16 = constp.tile([P, 3936], bf16)
            nc.sync.dma_start(out=cb16, in_=cb16_d[:])
            cb32 = constp.tile([P, 134], fp32)
            nc.sync.dma_start(out=cb32, in_=cb32_d[:])
            wqkv = cb16[:, 0:1536].rearrange("p (c x) -> p c x", c=2)
            wo = cb16[:, 1536:2048].rearrange("p (c x) -> p c x", c=2)
            seg = cb16[:, 2048:3776].rearrange(
                "p (m j i s) -> p m j i s", m=2, j=3, i=3)
            jsum = cb16[:, 3776:3808]
            iden = cb16[:, 3808:3936]
            jrep = cb32[0:32, 0:128]
            bqkv = cb32[:, 128:134]
            if use_bo:
                bo_rep = constp.tile([P, 2, D], fp32)
                nc.sync.dma_start(out=bo_rep,
                                  in_=bo_d[:].to_broadcast((P, 2, D)))
            eps_c = constp.tile([P, 1], fp32)
            nc.vector.memset(eps_c, EPS)
            mln3_c = constp.tile([P, 1], fp32)
            nc.vector.memset(mln3_c, -float(np.log(3.0)))
            if use_gamma:
                gam = constp.tile([P, D], bf16)
                nc.sync.dma_start(out=gam, in_=gam_d[:].to_broadcast((P, D)))
            if use_beta:
                bet = constp.tile([P, D], bf16)
                nc.sync.dma_start(out=bet, in_=bet_d[:].to_broadcast((P, D)))

            # greedy busy-tracking engine balancer (ns estimates)
            load = {"act": 0.0, "dve": 0.0, "pool": 0.0}

            def evac(dst, src, fd):
                # psum -> sbuf copy: ACT (fd+352)/1.2 vs DVE (120+fd/2)/0.96
                ca = (fd + 352) / 1.2
                cd = (120 + fd / 2) / 0.96
                if load["act"] + ca <= load["dve"] + cd:
                    load["act"] += ca
                    nc.scalar.copy(out=dst, in_=src)
                else:
                    load["dve"] += cd
                    nc.vector.tensor_copy(out=dst, in_=src)

            def tt(out, in0, in1, op, fd, psum=False):
                # bf16 TT: DVE 2x vs gpsimd ~1x (sbuf only)
                cd = ((120 if psum else 58) + fd / 2) / 0.96
                cp = (58 + fd) / 1.2
                if psum or load["dve"] + cd <= load["pool"] + cp:
                    load["dve"] += cd
                    nc.vector.tensor_tensor(out=out, in0=in0, in1=in1, op=op)
                else:
                    load["pool"] += cp
                    nc.gpsimd.tensor_tensor(out=out, in0=in0, in1=in1, op=op)

            def ts2(out, in0, s1, s2, fd):
                cd = (58 + fd / 4) / 0.96
                cp = (58 + fd / 2) / 1.2
                if load["dve"] + cd <= load["pool"] + cp:
                    load["dve"] += cd
                    nc.vector.tensor_scalar(out=out, in0=in0, scalar1=s1,
                                            scalar2=s2, op0=OP.subtract,
                                            op1=OP.mult)
                else:
                    load["pool"] += cp
                    nc.gpsimd.tensor_scalar(out=out, in0=in0, scalar1=s1,
                                            scalar2=s2, op0=OP.subtract,
                                            op1=OP.mult)

            def pe_transpose4(dst4, srcs):
                # 4x [128,128] transposes into one psum bank, single evac
                tp = tp_ps.tile([P, SUB, P], bf16, tag="tp")
                for s, sl in enumerate(srcs):
                    nc.tensor.transpose(tp[:, s, :], sl, iden)
                evac(dst4, tp, SUB * P)

            if NULL_KERNEL:
                zt = constp.tile([P, SUB, D], bf16)
                nc.vector.memset(zt, 0.0)
                for st in range(nst):
                    t0 = st * ST
                    dstn = out_d[t0:t0 + ST, :].rearrange("(s p) d -> p s d",
                                                          p=P)
                    nc.sync.dma_start(out=dstn, in_=zt)
                nst = 0
            for _rep in range(REPEAT if nst else 0):
              with tc.For_i(0, TOK, ST) as t0:
                # ---------- load + cast + PE-transpose ----------
                xb = []    # token-major bf16 [128, SUB, 256]
                xT = []    # feature-major bf16 [128, 2, 512]
                for i in range(3):
                    xbi = ldp.tile([P, SUB, D], bf16, tag=f"xb{i}")
                    src = xb_d[i, bass.ds(t0, ST), :].rearrange(
                        "(s p) d -> p s d", p=P)
                    nc.sync.dma_start(out=xbi, in_=src)
                    xb.append(xbi)
                    xTi = ldp.tile([P, 2, ST], bf16, tag=f"xT{i}")
                    if XT_FROM_DRAM:
                        nc.sync.dma_start(
                            out=xTi,
                            in_=xt_d[i, :, :, t0:t0 + ST].rearrange(
                                "c p t -> p c t"))
                    else:
                        for c in range(2):
                            pe_transpose4(
                                xTi[:, c, :],
                                [xbi[:, s, c * P:(c + 1) * P]
                                 for s in range(SUB)])
                    xT.append(xTi)

                # ---------- Q,K (W-stationary, feature-major) ----------
                qT, kT = [], []
                for i in range(3):
                    for pj in range(2):  # 0=q 1=k
                        dst = qkp.tile([P, 2, ST], bf16, tag=f"p{pj}m{i}")
                        for m in range(2):
                            ps = qk_ps.tile([P, ST], fp32, tag="qkps")
                            for c in range(2):
                                nc.tensor.matmul(
                                    ps,
                                    lhsT=wqkv[:, c,
                                              pj * D + m * P: pj * D + (m + 1) * P],
                                    rhs=xT[i][:, c, :],
                                    start=(c == 0), stop=(c == 1))
                            if 'evacqk' in SHRINK:
                                nc.vector.memset(dst[:, m, :], 0.1)
                            elif use_qkv_bias:
                                nc.scalar.activation(
                                    out=dst[:, m, :], in_=ps,
                                    func=AF.Identity,
                                    bias=bqkv[:, pj * 2 + m: pj * 2 + m + 1])
                            else:
                                evac(dst[:, m, :], ps, ST)
                        (qT if pj == 0 else kT).append(dst)

                # ---------- V (X^T-stationary, token-major) ----------
                vtok = []
                for i in range(3):
                    vt = midp.tile([P, SUB, D], bf16, tag=f"vtok{i}")
                    for spair in range(2):  # two sub-tiles per psum bank
                        ps = vo_ps.tile([P, 2, D], fp32, tag="vps")
                        for shalf in range(2):
                            s = spair * 2 + shalf
                            for c in range(2):
                                nc.tensor.matmul(
                                    ps[:, shalf, :],
                                    lhsT=xT[i][:, c, s * P:(s + 1) * P],
                                    rhs=wqkv[:, c, 2 * D:3 * D],
                                    start=(c == 0), stop=(c == 1))
                        evac(vt[:, spair * 2:spair * 2 + 2, :], ps, 2 * D)
                    vtok.append(vt)

                # ---------- scores ----------
                scp = sc_psp.tile([96, ST], fp32, tag="scmix")
                first = True
                for j in range(3):
                    for i in range(3):
                        pt = smallp.tile([P, 2, ST], bf16, tag="pmul")
                        if 'pmul' in SHRINK:
                            nc.vector.memset(pt, 0.25)
                        else:
                            tt(pt, qT[i], kT[j], OP.mult, 2 * ST)
                        for m in range(2):
                            last = (j == 2 and i == 2 and m == 1)
                            if 'seg' in SHRINK:
                                first = False
                                continue
                            nc.tensor.matmul(
                                scp, lhsT=seg[:, m, j, i, :], rhs=pt[:, m, :],
                                start=first, stop=last,
                                skip_group_check=True)
                            first = False
                if 'seg' in SHRINK:
                    nc.tensor.matmul(scp, lhsT=seg[:, 0, 0, 0, :],
                                     rhs=pt[:, 0, :], start=True, stop=True)

                # ---------- softmax ----------
                es = smallp.tile([P, ST], bf16, tag="es")
                nc.gpsimd.memset(es[96:128, :], 0.0)
                nc.scalar.activation(out=es[0:96, :], in_=scp[0:96, :],
                                     func=AF.Exp)
                zps = sc_psp.tile([32, ST], fp32, tag="scmix")
                nc.tensor.matmul(zps, lhsT=jsum[0:96, :], rhs=es[0:96, :],
                                 start=True, stop=True)
                zi = smallp.tile([32, ST], fp32, tag="zi")
                lnz = smallp.tile([32, ST], fp32, tag="lnz")
                nc.scalar.activation(out=lnz, in_=zps, func=AF.Ln)
                nc.scalar.activation(out=zi, in_=lnz, func=AF.Exp, scale=-1.0)
                zr = sc_psp.tile([P, ST], fp32, tag="scmix")
                nc.tensor.matmul(zr, lhsT=jrep, rhs=zi, start=True, stop=True)
                asb = smallp.tile([P, ST], bf16, tag="asb")
                tt(asb, es, zr, OP.mult, ST, psum=True)
                aT = smallp.tile([P, SUB, P], bf16, tag="aT")
                for s in range(SUB):
                    nc.sync.dma_start(out=aT[:, s, :],
                                      in_=asb[:, s * P:(s + 1) * P],
                                      transpose=True)

                # ---------- ctx ----------
                ctxT = []
                for i in range(3):
                    cx = ctxp.tile([P, SUB, D], bf16, tag=f"cx{i}")
                    tmp = ctxp.tile([P, SUB, D], bf16, tag="cxtmp")
                    cx4 = cx.rearrange("p s (h k) -> p s h k", h=H)
                    tmp4 = tmp.rearrange("p s (h k) -> p s h k", h=H)
                    if 'ctx' in SHRINK:
                        nc.vector.memset(cx, 0.5)
                    else:
                      for j in range(3):
                        asl = aT[:, :, 32 * j + 4 * i: 32 * j + 4 * i + 4]
                        abc = bass.AP(tensor=asl.tensor, offset=asl.offset,
                                      ap=[*asl.ap, [0, KD]])
                        v4 = vtok[j].rearrange("p s (h k) -> p s h k", h=H)
                        dst = cx4 if j == 0 else tmp4
                        tt(dst, v4, abc, OP.mult, SUB * D)
                        if j > 0:
                            tt(cx4, cx4, tmp4, OP.add, SUB * D)
                    cT = ctxp.tile([P, 2, ST], bf16, tag=f"cT{i}")
                    if 'ctxT' in SHRINK:
                        nc.vector.memset(cT, 0.2)
                    else:
                        for c in range(2):
                            pe_transpose4(
                                cT[:, c, :],
                                [cx[:, s, c * P:(c + 1) * P]
                                 for s in range(SUB)])
                    ctxT.append(cT)

                # ---------- O-proj (ctxT-stationary, token-major) + LN ------
                mvs = lnp.tile([P, 12, 2], fp32, tag="mvs")
                ys = []
                for i in range(3):
                    yi = lnp.tile([P, SUB, D], bf16, tag=f"y{i}")
                    for spair in range(2):
                        ops = vo_ps.tile([P, 2, D], fp32, tag="vps")
                        for shalf in range(2):
                            s = spair * 2 + shalf
                            for c in range(2):
                                nc.tensor.matmul(
                                    ops[:, shalf, :],
                                    lhsT=ctxT[i][:, c, s * P:(s + 1) * P],
                                    rhs=wo[:, c, :],
                                    start=(c == 0), stop=(c == 1))
                        if use_bo:
                            nc.vector.tensor_tensor(
                                out=ops, in0=ops, in1=bo_rep, op=OP.add)
                        ao = lnp.tile([P, 2, D], bf16, tag="ao")
                        evac(ao, ops, 2 * D)
                        for shalf in range(2):
                            s = spair * 2 + shalf
                            idx = i * SUB + s
                            if 'ln' in SHRINK:
                                continue
                            tt(yi[:, s, :], xb[i][:, s, :], ao[:, shalf, :],
                               OP.add, D)
                            st6 = lnp.tile([P, 6], fp32, tag="st6")
                            nc.vector.bn_stats(out=st6, in_=yi[:, s, :])
                            nc.vector.bn_aggr(out=mvs[:, idx, :], in_=st6)
                    ys.append(yi)

                # ---------- stats -> mu, istd/3 ----------
                if 'ln' in SHRINK:
                    otok = lnp.tile([P, SUB, D], bf16, tag="otok")
                    nc.vector.memset(otok, 0.0)
                    dst = out_d[bass.ds(t0, ST), :].rearrange("(s p) d -> p s d",
                                                              p=P)
                    nc.gpsimd.dma_start(out=dst, in_=otok)
                    continue
                lnv = lnp.tile([P, 12], fp32, tag="lnv")
                nc.scalar.activation(out=lnv, in_=mvs[:, :, 1], func=AF.Ln,
                                     bias=eps_c)
                ist = lnp.tile([P, 12], fp32, tag="ist")
                nc.scalar.activation(out=ist, in_=lnv, func=AF.Exp,
                                     scale=-0.5, bias=mln3_c)

                # ---------- apply + slot mean + store ----------
                otok = lnp.tile([P, SUB, D], bf16, tag="otok")
                for s in range(SUB):
                    n0 = lnp.tile([P, D], bf16, tag="n0")
                    n01 = lnp.tile([P, D], bf16, tag="n01")
                    n2 = lnp.tile([P, D], bf16, tag="n2")
                    idx = lambda i: i * SUB + s  # noqa: E731
                    ts2(n0, ys[0][:, s, :], mvs[:, idx(0), 0:1],
                        ist[:, idx(0):idx(0) + 1], D)
                    ts2(n2, ys[1][:, s, :], mvs[:, idx(1), 0:1],
                        ist[:, idx(1):idx(1) + 1], D)
                    tt(n01, n0, n2, OP.add, D)
                    ts2(n2, ys[2][:, s, :], mvs[:, idx(2), 0:1],
                        ist[:, idx(2):idx(2) + 1], D)
                    if use_gamma or use_beta:
                        fse = lnp.tile([P, D], bf16, tag="fse")
                        nc.vector.tensor_tensor(out=fse, in0=n01, in1=n2,
                                                op=OP.add)
                        if use_gamma:
                            nc.vector.tensor_tensor(out=fse, in0=fse, in1=gam,
                                                    op=OP.mult)
                        if use_beta:
                            nc.vector.tensor_tensor(out=otok[:, s, :], in0=fse,
                                                    in1=bet, op=OP.add)
                        else:
                            nc.vector.tensor_copy(out=otok[:, s, :], in_=fse)
                    else:
                        tt(otok[:, s, :], n01, n2, OP.add, D)
                dst = out_d[bass.ds(t0, ST), :].rearrange("(s p) d -> p s d",
                                                          p=P)
                nc.gpsimd.dma_start(out=dst, in_=otok)

    nc.compile()
    return nc


def _prep_weights(Wq, bq, Wk, bk, Wv, bv, Wo, bo, gamma, beta,
                  use_bo=False, use_gamma=False, use_beta=False):
    """Host-side packing of the small parameter tensors (packed layout)."""
    import ml_dtypes
    Wq2 = Wq.reshape(D, D)            # [d, (h k)]
    Wk2 = Wk.reshape(D, D)
    Wv2 = Wv.reshape(D, D)
    Wcat = np.concatenate([Wq2, Wk2, Wv2], axis=1)       # [256, 768]
    wqkv = Wcat.reshape(2, P, 3 * D).transpose(1, 0, 2).reshape(P, 1536)
    wo = Wo.reshape(D, D).reshape(2, P, D).transpose(1, 0, 2).reshape(P, 512)
    seg = np.zeros((P, 2, 3, 3, 96), np.float32)
    for m in range(2):
        for p in range(P):
            h = (m * P + p) // KD
            for j in range(3):
                for i in range(3):
                    seg[p, m, j, i, 32 * j + 4 * i + h] = 0.125
    jsum = np.zeros((P, 32), np.float32)
    for p in range(96):
        jsum[p, p % 32] = 1.0
    cb16 = np.concatenate(
        [wqkv, wo, seg.reshape(P, 1728), jsum, np.eye(P, dtype=np.float32)],
        axis=1).astype(ml_dtypes.bfloat16)                # [P, 3936]
    cb32 = np.zeros((P, 134), np.float32)
    for p in range(P):
        cb32[p % 32, p] = 1.0                             # jrep
    bcat = np.concatenate([bq.reshape(D), bk.reshape(D), bv.reshape(D)])
    cb32[:, 128:134] = bcat.reshape(3, 2, P).transpose(2, 0, 1).reshape(P, 6)
    out = {"cb16": cb16, "cb32": cb32}
    if use_bo:
        # v-bias folds into an effective output bias (softmax rows sum to 1):
        # ctx = sum_j a_ij (v_j + bv) = (sum_j a_ij v_j) + bv -> bv @ Wo + bo
        bo_eff = (bv.reshape(D) @ Wo.reshape(D, D) + bo.reshape(D))
        out["bo_t"] = bo_eff.reshape(1, D).astype(np.float32)
    if use_gamma:
        out["gam_t"] = gamma.reshape(1, D).astype(ml_dtypes.bfloat16)
    if use_beta:
        out["bet_t"] = beta.reshape(1, D).astype(ml_dtypes.bfloat16)
    return out


def kernel(**inputs):
    from concourse.bass_utils import run_bass_kernel_spmd

    xs = {k: np.asarray(inputs[k], np.float32)
          for k in ("x_tech", "x_sent", "x_fin")}
    params = {k: np.asarray(inputs[k], np.float32) for k in
              ("Wq", "bq", "Wk", "bk", "Wv", "bv", "Wo", "bo", "gamma", "beta")}

    use_qkv_bias = any(np.any(params[b]) for b in ("bq", "bk", "bv"))
    use_bo = bool(np.any(params["bo"])) or bool(np.any(params["bv"]))
    use_gamma = bool(np.any(params["gamma"] != 1.0))
    use_beta = bool(np.any(params["beta"]))

    import ml_dtypes
    import threading

    def fast_bf16(a):
        # round-to-nearest-even fp32 -> bf16 via integer ops (inputs finite)
        u = np.ascontiguousarray(a).view(np.uint32)
        r = (u + np.uint32(0x7FFF) + ((u >> np.uint32(16)) & np.uint32(1)))
        return (r >> np.uint32(16)).astype(np.uint16).view(ml_dtypes.bfloat16)

    # overlap the x casts (numpy, releases GIL) with the bass build
    xcast = {}
    cast_err = []

    def _do_cast():
        try:
            for name in ("x_tech", "x_sent", "x_fin"):
                xcast[name] = fast_bf16(xs[name]).reshape(B, N, D)
        except BaseException as e:  # noqa: BLE001
            cast_err.append(e)

    th = threading.Thread(target=_do_cast)
    th.start()

    TOK = (B // NCORES) * N
    key = (TOK, use_qkv_bias, use_bo, use_gamma, use_beta)
    if key not in _CACHE:
        _CACHE[key] = _build(*key)
    nc = _CACHE[key]

    wmap = _prep_weights(**params, use_bo=use_bo, use_gamma=use_gamma,
                         use_beta=use_beta)
    th.join()
    if cast_err:
        raise cast_err[0]
    in_maps = []
    for c in range(NCORES):
        m = dict(wmap)
        xbp = np.empty((3, TOK, D), ml_dtypes.bfloat16)
        for ii, name in enumerate(("x_tech", "x_sent", "x_fin")):
            xbp[ii] = xcast[name][c * (B // NCORES):
                                  (c + 1) * (B // NCORES)].reshape(TOK, D)
        m["xb_pre"] = xbp
        in_maps.append(m)

    res = run_bass_kernel_spmd(nc, in_maps, core_ids=list(range(NCORES)),
                               **_RUN_KWARGS)
    _LAST_RESULT[0] = res
    out = np.stack([np.asarray(r["out"]).reshape(B // NCORES, N, D)
                    for r in res.results])
    return out.reshape(B, N, D).astype(np.float32)



# revision 20
# speedup vs baseline: 2.9875x; 1.0778x over previous
"""Trainium2 Bass kernel for nn_CrossModalAttention.

Reference computation (per token t of B*N tokens):
  x = [x_tech_t; x_sent_t; x_fin_t]            # [3, 256]
  q/k/v = x @ W{q,k,v} + b                     # [3, 4, 64]
  scores = q k^T / 8 (per head), softmax over j
  ctx = attn @ v; attn_out = ctx @ Wo + bo     # [3, 256]
  y = x + attn_out; LayerNorm(d) per slot; mean over 3 slots -> [256]

Sharding: pure data-parallel over batch (64 -> 8 per core x 8 cores).

Per-core dataflow (TOK tokens, super-tiles of 512 = 4 sub-tiles of 128):
  - gpsimd cast-DMA HBM fp32 -> SBUF bf16, token-major xb [128,4,256]
  - PE transposes (identity matmul) -> xT feature-major [128,2,512]
  - Q,K: PE W-stationary -> feature-major psum; evac bf16 (ACT/DVE)
  - V: PE X^T-stationary -> token-major psum directly; evac bf16
  - scores: DVE/gpsimd mul P=Q_i^T*K_j^T; PE segment-reduce (indicator
    matmuls, 1/8 folded in) -> scores psum [96,512] rows=(j,i,h) 32-aligned
  - softmax: ACT exp; Z via PE indicator matmul; 1/Z = ACT exp(-ln Z);
    replicate via PE matmul; one DVE mul
  - a -> token-major via DMA-xbar transpose [128,4,128]
  - ctx: DVE/gpsimd tensor_tensor with 0-step free-dim broadcast of a over k
  - ctx -> PE-transpose -> ctxT; O-proj PE ctxT-stationary -> token-major psum
  - residual+LN: ACT evac, gpsimd residual add, DVE bn_stats/bn_aggr,
    istd via ACT Ln/Exp (exp table set shared), apply via tensor_scalar,
    slot-mean folded into istd (x 1/3)
"""

import numpy as np

D = 256
H = 4
KD = 64
EPS = 1e-6
B, N = 64, 1024
NCORES = 8
ST = 512          # tokens per super-tile
SUB = 4           # 128-token sub-tiles per super-tile
P = 128

_CACHE = {}
REPEAT = 1       # timing knob: loop the per-core program this many times
XT_FROM_DRAM = False  # False: PE-transpose xT on device
NULL_KERNEL = False   # timing: emit only output writes (dispatch baseline)
SHRINK = set()        # timing ablation: stages replaced by cheap memsets
_RUN_KWARGS = {}   # test harness may set e.g. {"trace": True}
_LAST_RESULT = [None]


def _build(TOK, use_qkv_bias, use_bo, use_gamma, use_beta):
    import concourse.bass as bass
    import concourse.bacc as bacc
    import concourse.mybir as mybir
    import concourse.tile as tile

    fp32 = mybir.dt.float32
    bf16 = mybir.dt.bfloat16
    AF = mybir.ActivationFunctionType
    OP = mybir.AluOpType

    nst = TOK // ST
    assert TOK % ST == 0

    nc = bacc.Bacc("TRN2", target_bir_lowering=False)

    # ---- DRAM I/O ----
    xb_d = nc.dram_tensor("xb_pre", [3, TOK, D], bf16, kind="ExternalInput")
    if XT_FROM_DRAM:
        xt_d = nc.dram_tensor("xt_pre", [3, 2, P, TOK], bf16,
                              kind="ExternalInput")
    # all bf16 consts packed into one tensor (fewer transfers):
    #   cols 0:1536 wqkv [P,2,768] | 1536:2048 wo [P,2,256]
    #   | 2048:3776 seg [P,2,3,3,96] | 3776:3808 jsum | 3808:3936 iden
    cb16_d = nc.dram_tensor("cb16", [P, 3936], bf16, kind="ExternalInput")
    # fp32 consts: cols 0:128 jrep (rows 0:32) | 128:134 bqkv
    cb32_d = nc.dram_tensor("cb32", [P, 134], fp32, kind="ExternalInput")
    if use_bo:
        bo_d = nc.dram_tensor("bo_t", [1, D], fp32, kind="ExternalInput")
    if use_gamma:
        gam_d = nc.dram_tensor("gam_t", [1, D], bf16, kind="ExternalInput")
    if use_beta:
        bet_d = nc.dram_tensor("bet_t", [1, D], bf16, kind="ExternalInput")
    out_d = nc.dram_tensor("out", [TOK, D], bf16, kind="ExternalOutput")

    with tile.TileContext(nc) as tc:
        with tc.tile_pool(name="const", bufs=1) as constp, \
             tc.tile_pool(name="ld", bufs=3) as ldp, \
             tc.tile_pool(name="qk", bufs=3) as qkp, \
             tc.tile_pool(name="mid", bufs=3) as midp, \
             tc.tile_pool(name="small", bufs=3) as smallp, \
             tc.tile_pool(name="ctxp", bufs=3) as ctxp, \
             tc.tile_pool(name="lnp", bufs=2) as lnp, \
             tc.tile_pool(name="qk_ps", bufs=2, space="PSUM") as qk_ps, \
             tc.tile_pool(name="vo_ps", bufs=2, space="PSUM") as vo_ps, \
             tc.tile_pool(name="sc_ps", bufs=2, space="PSUM") as sc_psp, \
             tc.tile_pool(name="tp_ps", bufs=2, space="PSUM") as tp_ps:

            # ---- constants (packed loads + views) ----
            cb16 = constp.tile([P, 3936], bf16)
            nc.sync.dma_start(out=cb16, in_=cb16_d[:])
            cb32 = constp.tile([P, 134], fp32)
            nc.sync.dma_start(out=cb32, in_=cb32_d[:])
            wqkv = cb16[:, 0:1536].rearrange("p (c x) -> p c x", c=2)
            wo = cb16[:, 1536:2048].rearrange("p (c x) -> p c x", c=2)
            seg = cb16[:, 2048:3776].rearrange(
                "p (m j i s) -> p m j i s", m=2, j=3, i=3)
            jsum = cb16[:, 3776:3808]
            iden = cb16[:, 3808:3936]
            jrep = cb32[0:32, 0:128]
            bqkv = cb32[:, 128:134]
            if use_bo:
                bo_rep = constp.tile([P, 2, D], fp32)
                nc.sync.dma_start(out=bo_rep,
                                  in_=bo_d[:].to_broadcast((P, 2, D)))
            eps_c = constp.tile([P, 1], fp32)
            nc.vector.memset(eps_c, EPS)
            mln3_c = constp.tile([P, 1], fp32)
            nc.vector.memset(mln3_c, -float(np.log(3.0)))
            if use_gamma:
                gam = constp.tile([P, D], bf16)
                nc.sync.dma_start(out=gam, in_=gam_d[:].to_broadcast((P, D)))
            if use_beta:
                bet = constp.tile([P, D], bf16)
                nc.sync.dma_start(out=bet, in_=bet_d[:].to_broadcast((P, D)))

            # greedy busy-tracking engine balancer (ns estimates)
            load = {"act": 0.0, "dve": 0.0, "pool": 0.0}

            def evac(dst, src, fd):
                # psum -> sbuf copy: ACT (fd+352)/1.2 vs DVE (120+fd/2)/0.96
                ca = (fd + 352) / 1.2
                cd = (120 + fd / 2) / 0.96
                if load["act"] + ca <= load["dve"] + cd:
                    load["act"] += ca
                    nc.scalar.copy(out=dst, in_=src)
                else:
                    load["dve"] += cd
                    nc.vector.tensor_copy(out=dst, in_=src)

            def tt(out, in0, in1, op, fd, psum=False):
                # bf16 TT: DVE 2x vs gpsimd ~1x (sbuf only)
                cd = ((120 if psum else 58) + fd / 2) / 0.96
                cp = (58 + fd) / 1.2
                if psum or load["dve"] + cd <= load["pool"] + cp:
                    load["dve"] += cd
                    nc.vector.tensor_tensor(out=out, in0=in0, in1=in1, op=op)
                else:
                    load["pool"] += cp
                    nc.gpsimd.tensor_tensor(out=out, in0=in0, in1=in1, op=op)

            def ts2(out, in0, s1, s2, fd):
                cd = (58 + fd / 4) / 0.96
                cp = (58 + fd / 2) / 1.2
                if load["dve"] + cd <= load["pool"] + cp:
                    load["dve"] += cd
                    nc.vector.tensor_scalar(out=out, in0=in0, scalar1=s1,
                                            scalar2=s2, op0=OP.subtract,
                                            op1=OP.mult)
                else:
                    load["pool"] += cp
                    nc.gpsimd.tensor_scalar(out=out, in0=in0, scalar1=s1,
                                            scalar2=s2, op0=OP.subtract,
                                            op1=OP.mult)

            def pe_transpose4(dst4, srcs):
                # 4x [128,128] transposes into one psum bank, single evac
                tp = tp_ps.tile([P, SUB, P], bf16, tag="tp")
                for s, sl in enumerate(srcs):
                    nc.tensor.transpose(tp[:, s, :], sl, iden)
                evac(dst4, tp, SUB * P)

            if NULL_KERNEL:
                zt = constp.tile([P, SUB, D], bf16)
                nc.vector.memset(zt, 0.0)
                for st in range(nst):
                    t0 = st * ST
                    dstn = out_d[t0:t0 + ST, :].rearrange("(s p) d -> p s d",
                                                          p=P)
                    nc.sync.dma_start(out=dstn, in_=zt)
                nst = 0
            for _rep in range(REPEAT if nst else 0):
              with tc.For_i(0, TOK, ST) as t0:
                # ---------- load + cast + PE-transpose ----------
                xb = []    # token-major bf16 [128, SUB, 256]
                xT = []    # feature-major bf16 [128, 2, 512]
                for i in range(3):
                    xbi = ldp.tile([P, SUB, D], bf16, tag=f"xb{i}")
                    src = xb_d[i, bass.ds(t0, ST), :].rearrange(
                        "(s p) d -> p s d", p=P)
                    nc.sync.dma_start(out=xbi, in_=src)
                    xb.append(xbi)
                    xTi = ldp.tile([P, 2, ST], bf16, tag=f"xT{i}")
                    if XT_FROM_DRAM:
                        nc.sync.dma_start(
                            out=xTi,
                            in_=xt_d[i, :, :, t0:t0 + ST].rearrange(
                                "c p t -> p c t"))
                    else:
                        for c in range(2):
                            pe_transpose4(
                                xTi[:, c, :],
                                [xbi[:, s, c * P:(c + 1) * P]
                                 for s in range(SUB)])
                    xT.append(xTi)

                # ---------- Q,K (W-stationary, feature-major) ----------
                qT, kT = [], []
                for i in range(3):
                    for pj in range(2):  # 0=q 1=k
                        dst = qkp.tile([P, 2, ST], bf16, tag=f"p{pj}m{i}")
                        for m in range(2):
                            ps = qk_ps.tile([P, ST], fp32, tag="qkps")
                            for c in range(2):
                                nc.tensor.matmul(
                                    ps,
                                    lhsT=wqkv[:, c,
                                              pj * D + m * P: pj * D + (m + 1) * P],
                                    rhs=xT[i][:, c, :],
                                    start=(c == 0), stop=(c == 1))
                            if 'evacqk' in SHRINK:
                                nc.vector.memset(dst[:, m, :], 0.1)
                            elif use_qkv_bias:
                                nc.scalar.activation(
                                    out=dst[:, m, :], in_=ps,
                                    func=AF.Identity,
                                    bias=bqkv[:, pj * 2 + m: pj * 2 + m + 1])
                            else:
                                evac(dst[:, m, :], ps, ST)
                        (qT if pj == 0 else kT).append(dst)

                # ---------- V (X^T-stationary, token-major) ----------
                vtok = []
                for i in range(3):
                    vt = midp.tile([P, SUB, D], bf16, tag=f"vtok{i}")
                    for spair in range(2):  # two sub-tiles per psum bank
                        ps = vo_ps.tile([P, 2, D], fp32, tag="vps")
                        for shalf in range(2):
                            s = spair * 2 + shalf
                            for c in range(2):
                                nc.tensor.matmul(
                                    ps[:, shalf, :],
                                    lhsT=xT[i][:, c, s * P:(s + 1) * P],
                                    rhs=wqkv[:, c, 2 * D:3 * D],
                                    start=(c == 0), stop=(c == 1))
                        evac(vt[:, spair * 2:spair * 2 + 2, :], ps, 2 * D)
                    vtok.append(vt)

                # ---------- scores ----------
                scp = sc_psp.tile([96, ST], fp32, tag="scmix")
                first = True
                for j in range(3):
                    for i in range(3):
                        pt = smallp.tile([P, 2, ST], bf16, tag="pmul")
                        if 'pmul' in SHRINK:
                            nc.vector.memset(pt, 0.25)
                        else:
                            tt(pt, qT[i], kT[j], OP.mult, 2 * ST)
                        for m in range(2):
                            last = (j == 2 and i == 2 and m == 1)
                            if 'seg' in SHRINK:
                                first = False
                                continue
                            nc.tensor.matmul(
                                scp, lhsT=seg[:, m, j, i, :], rhs=pt[:, m, :],
                                start=first, stop=last,
                                skip_group_check=True)
                            first = False
                if 'seg' in SHRINK:
                    nc.tensor.matmul(scp, lhsT=seg[:, 0, 0, 0, :],
                                     rhs=pt[:, 0, :], start=True, stop=True)

                # ---------- softmax ----------
                es = smallp.tile([P, ST], bf16, tag="es")
                nc.gpsimd.memset(es[96:128, :], 0.0)
                nc.scalar.activation(out=es[0:96, :], in_=scp[0:96, :],
                                     func=AF.Exp)
                zps = sc_psp.tile([32, ST], fp32, tag="scmix")
                nc.tensor.matmul(zps, lhsT=jsum[0:96, :], rhs=es[0:96, :],
                                 start=True, stop=True)
                zi = smallp.tile([32, ST], fp32, tag="zi")
                lnz = smallp.tile([32, ST], fp32, tag="lnz")
                nc.scalar.activation(out=lnz, in_=zps, func=AF.Ln)
                nc.scalar.activation(out=zi, in_=lnz, func=AF.Exp, scale=-1.0)
                zr = sc_psp.tile([P, ST], fp32, tag="scmix")
                nc.tensor.matmul(zr, lhsT=jrep, rhs=zi, start=True, stop=True)
                asb = smallp.tile([P, ST], bf16, tag="asb")
                tt(asb, es, zr, OP.mult, ST, psum=True)
                aT = smallp.tile([P, SUB, P], bf16, tag="aT")
                for s in range(SUB):
                    nc.sync.dma_start(out=aT[:, s, :],
                                      in_=asb[:, s * P:(s + 1) * P],
                                      transpose=True)

                # ---------- ctx ----------
                ctxT = []
                for i in range(3):
                    cx = ctxp.tile([P, SUB, D], bf16, tag=f"cx{i}")
                    tmp = ctxp.tile([P, SUB, D], bf16, tag="cxtmp")
                    cx4 = cx.rearrange("p s (h k) -> p s h k", h=H)
                    tmp4 = tmp.rearrange("p s (h k) -> p s h k", h=H)
                    if 'ctx' in SHRINK:
                        nc.vector.memset(cx, 0.5)
                    else:
                      for j in range(3):
                        asl = aT[:, :, 32 * j + 4 * i: 32 * j + 4 * i + 4]
                        abc = bass.AP(tensor=asl.tensor, offset=asl.offset,
                                      ap=[*asl.ap, [0, KD]])
                        v4 = vtok[j].rearrange("p s (h k) -> p s h k", h=H)
                        dst = cx4 if j == 0 else tmp4
                        tt(dst, v4, abc, OP.mult, SUB * D)
                        if j > 0:
                            tt(cx4, cx4, tmp4, OP.add, SUB * D)
                    cT = ctxp.tile([P, 2, ST], bf16, tag=f"cT{i}")
                    if 'ctxT' in SHRINK:
                        nc.vector.memset(cT, 0.2)
                    else:
                        for c in range(2):
                            pe_transpose4(
                                cT[:, c, :],
                                [cx[:, s, c * P:(c + 1) * P]
                                 for s in range(SUB)])
                    ctxT.append(cT)

                # ---------- O-proj (ctxT-stationary, token-major) + LN ------
                mvs = lnp.tile([P, 12, 2], fp32, tag="mvs")
                ys = []
                for i in range(3):
                    yi = lnp.tile([P, SUB, D], bf16, tag=f"y{i}")
                    for spair in range(2):
                        ops = vo_ps.tile([P, 2, D], fp32, tag="vps")
                        for shalf in range(2):
                            s = spair * 2 + shalf
                            for c in range(2):
                                nc.tensor.matmul(
                                    ops[:, shalf, :],
                                    lhsT=ctxT[i][:, c, s * P:(s + 1) * P],
                                    rhs=wo[:, c, :],
                                    start=(c == 0), stop=(c == 1))
                        if use_bo:
                            nc.vector.tensor_tensor(
                                out=ops, in0=ops, in1=bo_rep, op=OP.add)
                        ao = lnp.tile([P, 2, D], bf16, tag="ao")
                        evac(ao, ops, 2 * D)
                        for shalf in range(2):
                            s = spair * 2 + shalf
                            idx = i * SUB + s
                            if 'ln' in SHRINK:
                                continue
                            tt(yi[:, s, :], xb[i][:, s, :], ao[:, shalf, :],
                               OP.add, D)
                            st6 = lnp.tile([P, 6], fp32, tag="st6")
                            nc.vector.bn_stats(out=st6, in_=yi[:, s, :])
                            nc.vector.bn_aggr(out=mvs[:, idx, :], in_=st6)
                    ys.append(yi)

                # ---------- stats -> mu, istd/3 ----------
                if 'ln' in SHRINK:
                    otok = lnp.tile([P, SUB, D], bf16, tag="otok")
                    nc.vector.memset(otok, 0.0)
                    dst = out_d[bass.ds(t0, ST), :].rearrange("(s p) d -> p s d",
                                                              p=P)
                    nc.gpsimd.dma_start(out=dst, in_=otok)
                    continue
                lnv = lnp.tile([P, 12], fp32, tag="lnv")
                nc.scalar.activation(out=lnv, in_=mvs[:, :, 1], func=AF.Ln,
                                     bias=eps_c)
                ist = lnp.tile([P, 12], fp32, tag="ist")
                nc.scalar.activation(out=ist, in_=lnv, func=AF.Exp,
                                     scale=-0.5, bias=mln3_c)

                # ---------- apply + slot mean + store ----------
                otok = lnp.tile([P, SUB, D], bf16, tag="otok")
                for s in range(SUB):
                    n0 = lnp.tile([P, D], bf16, tag="n0")
                    n01 = lnp.tile([P, D], bf16, tag="n01")
                    n2 = lnp.tile([P, D], bf16, tag="n2")
                    idx = lambda i: i * SUB + s  # noqa: E731
                    ts2(n0, ys[0][:, s, :], mvs[:, idx(0), 0:1],
                        ist[:, idx(0):idx(0) + 1], D)
                    ts2(n2, ys[1][:, s, :], mvs[:, idx(1), 0:1],
                        ist[:, idx(1):idx(1) + 1], D)
                    tt(n01, n0, n2, OP.add, D)
                    ts2(n2, ys[2][:, s, :], mvs[:, idx(2), 0:1],
                        ist[:, idx(2):idx(2) + 1], D)
                    if use_gamma or use_beta:
                        fse = lnp.tile([P, D], bf16, tag="fse")
                        nc.vector.tensor_tensor(out=fse, in0=n01, in1=n2,
                                                op=OP.add)
                        if use_gamma:
                            nc.vector.tensor_tensor(out=fse, in0=fse, in1=gam,
                                                    op=OP.mult)
                        if use_beta:
                            nc.vector.tensor_tensor(out=otok[:, s, :], in0=fse,
                                                    in1=bet, op=OP.add)
                        else:
                            nc.vector.tensor_copy(out=otok[:, s, :], in_=fse)
                    else:
                        tt(otok[:, s, :], n01, n2, OP.add, D)
                dst = out_d[bass.ds(t0, ST), :].rearrange("(s p) d -> p s d",
                                                          p=P)
                nc.gpsimd.dma_start(out=dst, in_=otok)

    nc.compile()
    return nc


def _prep_weights(Wq, bq, Wk, bk, Wv, bv, Wo, bo, gamma, beta,
                  use_bo=False, use_gamma=False, use_beta=False):
    """Host-side packing of the small parameter tensors (packed layout)."""
    import ml_dtypes
    Wq2 = Wq.reshape(D, D)            # [d, (h k)]
    Wk2 = Wk.reshape(D, D)
    Wv2 = Wv.reshape(D, D)
    Wcat = np.concatenate([Wq2, Wk2, Wv2], axis=1)       # [256, 768]
    wqkv = Wcat.reshape(2, P, 3 * D).transpose(1, 0, 2).reshape(P, 1536)
    wo = Wo.reshape(D, D).reshape(2, P, D).transpose(1, 0, 2).reshape(P, 512)
    seg = np.zeros((P, 2, 3, 3, 96), np.float32)
    for m in range(2):
        for p in range(P):
            h = (m * P + p) // KD
            for j in range(3):
                for i in range(3):
                    seg[p, m, j, i, 32 * j + 4 * i + h] = 0.125
    jsum = np.zeros((P, 32), np.float32)
    for p in range(96):
        jsum[p, p % 32] = 1.0
    cb16 = np.concatenate(
        [wqkv, wo, seg.reshape(P, 1728), jsum, np.eye(P, dtype=np.float32)],
        axis=1).astype(ml_dtypes.bfloat16)                # [P, 3936]
    cb32 = np.zeros((P, 134), np.float32)
    for p in range(P):
        cb32[p % 32, p] = 1.0                             # jrep
    bcat = np.concatenate([bq.reshape(D), bk.reshape(D), bv.reshape(D)])
    cb32[:, 128:134] = bcat.reshape(3, 2, P).transpose(2, 0, 1).reshape(P, 6)
    out = {"cb16": cb16, "cb32": cb32}
    if use_bo:
        # v-bias folds into an effective output bias (softmax rows sum to 1):
        # ctx = sum_j a_ij (v_j + bv) = (sum_j a_ij v_j) + bv -> bv @ Wo + bo
        bo_eff = (bv.reshape(D) @ Wo.reshape(D, D) + bo.reshape(D))
        out["bo_t"] = bo_eff.reshape(1, D).astype(np.float32)
    if use_gamma:
        out["gam_t"] = gamma.reshape(1, D).astype(ml_dtypes.bfloat16)
    if use_beta:
        out["bet_t"] = beta.reshape(1, D).astype(ml_dtypes.bfloat16)
    return out


def kernel(**inputs):
    from concourse.bass_utils import run_bass_kernel_spmd

    xs = {k: np.asarray(inputs[k], np.float32)
          for k in ("x_tech", "x_sent", "x_fin")}
    params = {k: np.asarray(inputs[k], np.float32) for k in
              ("Wq", "bq", "Wk", "bk", "Wv", "bv", "Wo", "bo", "gamma", "beta")}

    use_qkv_bias = any(np.any(params[b]) for b in ("bq", "bk", "bv"))
    use_bo = bool(np.any(params["bo"])) or bool(np.any(params["bv"]))
    use_gamma = bool(np.any(params["gamma"] != 1.0))
    use_beta = bool(np.any(params["beta"]))

    import ml_dtypes
    import threading

    # bf16 by truncation = high uint16 of each little-endian fp32; a single
    # strided copy per modality.  The ~0.5-ulp magnitude shrink is uniform
    # and mostly cancels through the LayerNorm.
    def trunc_hi(a):
        return np.ascontiguousarray(a).view(np.uint16).reshape(
            B, N, D, 2)[..., 1]

    # overlap the x casts (numpy, releases GIL) with the bass build
    xcast = {}
    cast_err = []

    def _do_cast():
        try:
            for name in ("x_tech", "x_sent", "x_fin"):
                xcast[name] = trunc_hi(xs[name])
        except BaseException as e:  # noqa: BLE001
            cast_err.append(e)

    th = threading.Thread(target=_do_cast)
    th.start()

    TOK = (B // NCORES) * N
    key = (TOK, use_qkv_bias, use_bo, use_gamma, use_beta)
    if key not in _CACHE:
        _CACHE[key] = _build(*key)
    nc = _CACHE[key]

    wmap = _prep_weights(**params, use_bo=use_bo, use_gamma=use_gamma,
                         use_beta=use_beta)
    th.join()
    if cast_err:
        raise cast_err[0]
    in_maps = []
    for c in range(NCORES):
        m = dict(wmap)
        xbp = np.empty((3, TOK, D), np.uint16)
        for ii, name in enumerate(("x_tech", "x_sent", "x_fin")):
            xbp[ii] = xcast[name][c * (B // NCORES):
                                  (c + 1) * (B // NCORES)].reshape(TOK, D)
        m["xb_pre"] = xbp.view(ml_dtypes.bfloat16)
        in_maps.append(m)

    res = run_bass_kernel_spmd(nc, in_maps, core_ids=list(range(NCORES)),
                               **_RUN_KWARGS)
    _LAST_RESULT[0] = res
    out = np.stack([np.asarray(r["out"]).reshape(B // NCORES, N, D)
                    for r in res.results])
    return out.reshape(B, N, D).astype(np.float32)

